# revision 5
# baseline (speedup 1.0000x reference)
"""MiniMax Lightning Attention on 8 Trainium2 NeuronCores.

Sharding: sequence-parallel. Core c handles batch c//4, token chunk
(c%4)*1024..+1024 (4 blocks of 256). The per-block decay-state recurrence
crosses chunk boundaries; each core computes its local per-chunk decay-
weighted KV summary E, an AllGather shares the 8 summaries, and each core
reconstructs its chunk-start state as a decay-weighted sum.

All matmuls run as fp32r (full-rate fp32 on the PE at N>=256).
"""

import numpy as np

import concourse.bacc as bacc
import concourse.mybir as mybir
import concourse.tile as tile
from concourse.bass_utils import run_bass_kernel_spmd
from concourse.masks import make_identity

AF = mybir.ActivationFunctionType
ALU = mybir.AluOpType
F32 = mybir.dt.float32
F32R = mybir.dt.float32r

H = 32
D = 64
BS = 256
HID = 2048
B = 2
S = 4096
NC = 8
T = S // 4            # tokens per core (1024)
NCH = T // 128        # 8 token chunks of 128
NBLK = T // BS        # 4 blocks per core
LAYER_IDX = 0
NUM_LAYERS = 32
EPS = 1e-5


def _decay():
    base = 1.0 / 2.0 ** (8.0 / H)
    rate = base ** (np.arange(H, dtype=np.float64) + 1.0)
    factor = 1.0 - LAYER_IDX / (NUM_LAYERS - 1 + 1e-5) + 1e-5
    slope = rate * factor                                  # (H,)
    r = np.arange(BS, dtype=np.float64) + 1.0
    qd = np.exp(-slope[:, None] * r[None, :])              # (H, BS) query decay
    kd = np.exp(-slope[:, None] * (BS - r[None, :]))       # (H, BS) key decay
    ij = r[:, None] - r[None, :]                           # i - j
    dd = np.where(
        ij[None] >= 0, np.exp(-slope[:, None, None] * ij[None]), 0.0
    )                                                      # (H, BS_i, BS_j)
    bd = np.exp(-slope * BS)                               # (H,) block decay
    return slope, qd, kd, dd, bd


def _build_nc():
    nc = bacc.Bacc(num_devices=NC)
    hsT = nc.declare_dram_parameter("hsT", [HID, T], F32R, isOutput=False)
    wqkT = nc.declare_dram_parameter("wqkT", [HID, 2 * H * D], F32R, isOutput=False)
    wvT = nc.declare_dram_parameter("wvT", [HID, H * D], F32R, isOutput=False)
    gwT = nc.declare_dram_parameter("gwT", [HID, HID], F32R, isOutput=False)
    owT = nc.declare_dram_parameter("owT", [H * D, HID], F32R, isOutput=False)
    ddm = nc.declare_dram_parameter("ddm", [H, 2, 128, BS], F32, isOutput=False)
    qdm = nc.declare_dram_parameter("qdm", [H, D, BS], F32, isOutput=False)
    kdm = nc.declare_dram_parameter("kdm", [128, 2 * H], F32, isOutput=False)
    nw = nc.declare_dram_parameter("nw", [128, 16], F32, isOutput=False)
    swm = nc.declare_dram_parameter("swm", [D, H * NC], F32, isOutput=False)
    out = nc.declare_dram_parameter("out", [T, HID], F32, isOutput=True)

    qk_spill = nc.dram_tensor("qk_spill", [H, 2, D, T], F32R)
    gate_spill = nc.dram_tensor("gate_spill", [16, 128, T], F32)
    attn_spill = nc.dram_tensor("attn_spill", [16, 128, T], F32R)
    c_dram = nc.dram_tensor("c_dram", [H, D, NBLK * D], F32)
    eloc = nc.dram_tensor("eloc", [H, D, D], F32)
    egath = nc.dram_tensor("egath", [NC, H, D, D], F32, addr_space="Shared")
    ssq_rt = nc.dram_tensor("ssq_rt", [T], F32)

    bd_f = [float(x) for x in _decay()[4]]

    with tile.TileContext(nc, pool_alloc_mode="queue") as tc:
        # ---- constants + resident tensors -------------------------------
        ident, free_ident = tc.tile([128, 128], F32, name="ident")
        make_identity(nc, ident[:])
        ones_f, free_ones_f = tc.tile([128, 1], F32, name="ones_f")
        nc.vector.memset(ones_f[:], 1.0)
        ones, free_ones = tc.tile([128, 1], F32R, name="ones")
        nc.scalar.copy(ones[:], ones_f[:])
        eps_sb, free_eps = tc.tile([128, 1], F32, name="eps_sb")
        nc.vector.memset(eps_sb[:], EPS)
        nw_sb, free_nw = tc.tile([128, 16], F32, name="nw_sb")
        nc.sync.dma_start(nw_sb[:], nw[:])
        kdm_sb, free_kdm = tc.tile([128, 2 * H], F32, name="kdm_sb")
        nc.sync.dma_start(kdm_sb[:], kdm[:])
        swm_sb, free_swm = tc.tile([D, H * NC], F32, name="swm_sb")
        nc.sync.dma_start(swm_sb[:], swm[:])

        V_sb, free_V = tc.tile([128, NCH, H * D], F32R, name="V_sb")
        xT, free_xT = tc.tile([128, 16, T], F32R, name="xT")
        for k in range(16):
            nc.sync.dma_start(xT[:, k, :], hsT[k * 128 : (k + 1) * 128, :])

        # ---- phase V: value projection (tok-major, all heads) -----------
        with tc.tile_pool(name="wv_p", bufs=3) as wv_p, tc.tile_pool(
            name="ps_v", bufs=1, space="PSUM"
        ) as ps_v:
            for n in range(4):
                pv = [
                    ps_v.tile([128, 512], F32, name=f"pv{m}") for m in range(NCH)
                ]
                for k in range(16):
                    wv_t = wv_p.tile([128, 512], F32R, name="wv_t")
                    nc.sync.dma_start(
                        wv_t[:], wvT[k * 128 : (k + 1) * 128, n * 512 : (n + 1) * 512]
                    )
                    for m in range(NCH):
                        nc.tensor.matmul(
                            pv[m][:],
                            xT[:, k, m * 128 : (m + 1) * 128],
                            wv_t[:],
                            start=(k == 0),
                            stop=(k == 15),
                        )
                for m in range(NCH):
                    nc.scalar.activation(
                        V_sb[:, m, n * 512 : (n + 1) * 512], pv[m][:], AF.Silu
                    )

        # ---- phase QK: q/k projection (dim-major per head) + contribs ---
        with tc.tile_pool(name="wqk_p", bufs=2) as wqk_p, tc.tile_pool(
            name="qk_p", bufs=2
        ) as qk_p, tc.tile_pool(name="tok_p", bufs=2) as tok_p, tc.tile_pool(
            name="ce_p", bufs=2
        ) as ce_p, tc.tile_pool(
            name="ps_qk", bufs=2, space="PSUM"
        ) as ps_qk, tc.tile_pool(
            name="ps_t", bufs=2, space="PSUM"
        ) as ps_t, tc.tile_pool(
            name="ps_c", bufs=2, space="PSUM"
        ) as ps_c:
            for h in range(H):
                wqk_t = wqk_p.tile([128, 16, 128], F32R, name="wqk_t")
                nc.sync.dma_start(
                    wqk_t[:],
                    wqkT[:, h * 128 : (h + 1) * 128].rearrange(
                        "(ko p) m -> p ko m", p=128
                    ),
                )
                pqk = ps_qk.tile([128, 2, 512], F32, name="pqk")
                for n in range(2):
                    for k in range(16):
                        nc.tensor.matmul(
                            pqk[:, n, :],
                            wqk_t[:, k, :],
                            xT[:, k, n * 512 : (n + 1) * 512],
                            start=(k == 0),
                            stop=(k == 15),
                        )
                qT_t = qk_p.tile([D, T], F32R, name="qT_t")
                kT_t = qk_p.tile([D, T], F32R, name="kT_t")
                nc.scalar.activation(
                    qT_t[:], pqk[0:D].rearrange("p n f -> p (n f)"), AF.Silu
                )
                nc.scalar.activation(
                    kT_t[:], pqk[D:128].rearrange("p n f -> p (n f)"), AF.Silu
                )
                nc.sync.dma_start(qk_spill[h, 0], qT_t[:])
                nc.sync.dma_start(qk_spill[h, 1], kT_t[:])

                # k back to tok-major via PE transpose
                k_tok = tok_p.tile([128, NCH, D], F32R, name="k_tok")
                for m in range(NCH):
                    pst = ps_t.tile([128, D], F32, name="pst")
                    nc.tensor.transpose(
                        pst[:],
                        kT_t[:, m * 128 : (m + 1) * 128].bitcast(F32),
                        ident[0:D, 0:D],
                    )
                    nc.scalar.copy(k_tok[:, m, :], pst[:])
                # v scaled by key-decay
                v_kd = tok_p.tile([128, NCH, D], F32R, name="v_kd")
                for m in range(NCH):
                    nc.vector.tensor_scalar_mul(
                        v_kd[:, m, :],
                        V_sb[:, m, h * D : (h + 1) * D],
                        kdm_sb[:, 2 * h + (m % 2) : 2 * h + (m % 2) + 1],
                    )
                # block contributions C_jb = (k*kd)^T v and chunk summary E
                c_st = ce_p.tile([D, NBLK, D], F32, name="c_st")
                for jb in range(NBLK):
                    pc = ps_c.tile([D, D], F32, name="pc")
                    for half in range(2):
                        m = 2 * jb + half
                        nc.tensor.matmul(
                            pc[:],
                            k_tok[:, m, :],
                            v_kd[:, m, :],
                            start=(half == 0),
                            stop=(half == 1),
                        )
                    nc.scalar.copy(c_st[:, jb, :], pc[:])
                nc.sync.dma_start(c_dram[h], c_st[:].rearrange("d b e -> d (b e)"))
                e_t = ce_p.tile([D, D], F32, name="e_t")
                nc.vector.tensor_copy(e_t[:], c_st[:, 0, :])
                for jb in range(1, NBLK):
                    nc.vector.scalar_tensor_tensor(
                        e_t[:], e_t[:], bd_f[h], c_st[:, jb, :], ALU.mult, ALU.add
                    )
                nc.sync.dma_start(eloc[h], e_t[:])

        # ---- collective: share per-chunk KV summaries -------------------
        nc.gpsimd.collective_compute(
            "AllGather",
            ALU.bypass,
            replica_groups=[list(range(NC))],
            ins=[eloc[:]],
            outs=[egath[:]],
        )

        # ---- phase G: gate projection (overlaps the collective) ---------
        with tc.tile_pool(name="gw_p", bufs=2) as gw_p, tc.tile_pool(
            name="go_p", bufs=2
        ) as go_p, tc.tile_pool(name="ps_g", bufs=2, space="PSUM") as ps_g:
            for m in range(16):
                gw_t = gw_p.tile([128, 16, 128], F32R, name="gw_t")
                nc.sync.dma_start(
                    gw_t[:],
                    gwT[:, m * 128 : (m + 1) * 128].rearrange(
                        "(ko p) g -> p ko g", p=128
                    ),
                )
                for n in range(2):
                    pg = ps_g.tile([128, 512], F32, name="pg")
                    for k in range(16):
                        nc.tensor.matmul(
                            pg[:],
                            gw_t[:, k, :],
                            xT[:, k, n * 512 : (n + 1) * 512],
                            start=(k == 0),
                            stop=(k == 15),
                        )
                    go_t = go_p.tile([128, 512], F32, name="go_t")
                    nc.scalar.activation(go_t[:], pg[:], AF.Sigmoid)
                    nc.sync.dma_start(
                        gate_spill[m, :, n * 512 : (n + 1) * 512], go_t[:]
                    )
        free_xT()

        # ---- phase A: attention per head --------------------------------
        with tc.tile_pool(name="aq_p", bufs=3) as aq_p, tc.tile_pool(
            name="am_p", bufs=2
        ) as am_p, tc.tile_pool(name="ss_p", bufs=3) as ss_p, tc.tile_pool(
            name="ys_p", bufs=3
        ) as ys_p, tc.tile_pool(
            name="ps_aw", bufs=2, space="PSUM"
        ) as ps_aw, tc.tile_pool(
            name="ps_ys", bufs=2, space="PSUM"
        ) as ps_ys:
            for h in range(H):
                qT_a = aq_p.tile([D, T], F32R, name="qT_a")
                kT_a = aq_p.tile([D, T], F32R, name="kT_a")
                nc.sync.dma_start(qT_a[:], qk_spill[h, 0])
                nc.sync.dma_start(kT_a[:], qk_spill[h, 1])
                dd_t = am_p.tile([128, 2, BS], F32, name="dd_t")
                nc.sync.dma_start(dd_t[:], ddm[h].rearrange("c p i -> p c i"))
                qd_t = am_p.tile([D, BS], F32, name="qd_t")
                nc.sync.dma_start(qd_t[:], qdm[h])
                eg_t = am_p.tile([D, NC, D], F32, name="eg_t")
                nc.sync.dma_start(eg_t[:], egath[:, h, :, :].rearrange("c d e -> d c e"))
                c_a = am_p.tile([D, NBLK, D], F32, name="c_a")
                nc.sync.dma_start(c_a[:], c_dram[h].rearrange("d (b e) -> d b e", b=NBLK))

                qdq = aq_p.tile([D, NBLK, BS], F32R, name="qdq")
                for jb in range(NBLK):
                    nc.vector.tensor_mul(
                        qdq[:, jb, :],
                        qT_a[:, jb * BS : (jb + 1) * BS].bitcast(F32),
                        qd_t[:],
                    )
                ss = ss_p.tile([D, D], F32R, name="ss")
                nc.vector.tensor_scalar_mul(
                    ss[:], eg_t[:, 0, :], swm_sb[:, h * NC : h * NC + 1]
                )
                for cc in range(1, NC):
                    nc.vector.scalar_tensor_tensor(
                        ss[:],
                        eg_t[:, cc, :],
                        swm_sb[:, h * NC + cc : h * NC + cc + 1],
                        ss[:],
                        ALU.mult,
                        ALU.add,
                    )
                for jb in range(NBLK):
                    paw = ps_aw.tile([128, 2, BS], F32, name="paw")
                    for jc in range(2):
                        nc.tensor.matmul(
                            paw[:, jc, :],
                            kT_a[:, jb * BS + jc * 128 : jb * BS + (jc + 1) * 128],
                            qT_a[:, jb * BS : (jb + 1) * BS],
                            start=True,
                            stop=True,
                        )
                    awm = ys_p.tile([128, 2, BS], F32R, name="awm")
                    nc.vector.tensor_mul(awm[:], paw[:], dd_t[:])
                    pys = ps_ys.tile([D, BS], F32, name="pys")
                    nc.tensor.matmul(
                        pys[:], ss[:], qdq[:, jb, :], start=True, stop=False
                    )
                    for jc in range(2):
                        nc.tensor.matmul(
                            pys[:],
                            V_sb[:, 2 * jb + jc, h * D : (h + 1) * D],
                            awm[:, jc, :],
                            start=False,
                            stop=(jc == 1),
                        )
                    ys_t = ys_p.tile([D, BS], F32R, name="ys_t")
                    nc.scalar.copy(ys_t[:], pys[:])
                    nc.sync.dma_start(
                        attn_spill[h // 2, (h % 2) * D : (h % 2 + 1) * D,
                                   jb * BS : (jb + 1) * BS],
                        ys_t[:],
                    )
                    if jb < NBLK - 1:
                        ss2 = ss_p.tile([D, D], F32R, name="ss")
                        nc.vector.scalar_tensor_tensor(
                            ss2[:], ss[:], bd_f[h], c_a[:, jb, :], ALU.mult, ALU.add
                        )
                        ss = ss2
        free_V()

        # ---- phase F: rmsnorm + gate + output projection ----------------
        gate_sb, free_gate = tc.tile([128, 16, T], F32, name="gate_sb")
        for c in range(16):
            nc.sync.dma_start(gate_sb[:, c, :], gate_spill[c])
        with tc.tile_pool(name="sq_p", bufs=2) as sq_p, tc.tile_pool(
            name="an_p", bufs=3
        ) as an_p:
          with tc.tile_pool(name="ps_sq", bufs=1, space="PSUM") as ps_sq:
            ssq0 = ps_sq.tile([1, 512], F32, name="ssq0")
            ssq1 = ps_sq.tile([1, 512], F32, name="ssq1")
            for c in range(16):
                at = an_p.tile([128, T], F32R, name="at")
                nc.sync.dma_start(at[:], attn_spill[c])
                sq = sq_p.tile([128, T], F32R, name="sq")
                nc.scalar.activation(sq[:], at[:].bitcast(F32), AF.Square)
                for half in range(2):
                    nc.tensor.matmul(
                        [ssq0, ssq1][half][:],
                        ones[:],
                        sq[:, half * 512 : (half + 1) * 512],
                        start=(c == 0),
                        stop=(c == 15),
                        skip_group_check=True,
                    )
                nc.vector.scalar_tensor_tensor(
                    gate_sb[:, c, :].bitcast(F32R),
                    at[:].bitcast(F32),
                    nw_sb[:, c : c + 1],
                    gate_sb[:, c, :],
                    ALU.mult,
                    ALU.mult,
                )
            ssq_sb = sq_p.tile([1, T], F32, name="ssq_sb")
            nc.vector.tensor_copy(ssq_sb[:, 0:512], ssq0[:])
            nc.vector.tensor_copy(ssq_sb[:, 512:1024], ssq1[:])
            nc.sync.dma_start(ssq_rt[:], ssq_sb[:])
          if True:
            ns_l = sq_p.tile([128, NCH], F32, name="ns_l")
            nc.sync.dma_start(ns_l[:], ssq_rt.rearrange("(c p) -> p c", p=128))
            ns_t = sq_p.tile([128, NCH], F32, name="ns_t")
            nc.scalar.activation(
                ns_t[:], ns_l[:], AF.Sqrt, bias=eps_sb[:, 0:1], scale=1.0 / (H * D)
            )
            ns_sb = sq_p.tile([128, NCH], F32, name="ns_sb")
            nc.vector.reciprocal(ns_sb[:], ns_t[:])

            with tc.tile_pool(name="ow_p", bufs=3) as ow_p, tc.tile_pool(
                name="oo_p", bufs=3
            ) as oo_p, tc.tile_pool(name="ps_o", bufs=1, space="PSUM") as ps_o:
                for n in range(4):
                    po = [
                        ps_o.tile([128, 512], F32, name=f"po{m}") for m in range(NCH)
                    ]
                    for k in range(16):
                        ow_t = ow_p.tile([128, 512], F32R, name="ow_t")
                        nc.sync.dma_start(
                            ow_t[:],
                            owT[k * 128 : (k + 1) * 128, n * 512 : (n + 1) * 512],
                        )
                        for m in range(NCH):
                            nc.tensor.matmul(
                                po[m][:],
                                gate_sb[:, k, m * 128 : (m + 1) * 128].bitcast(F32R),
                                ow_t[:],
                                start=(k == 0),
                                stop=(k == 15),
                            )
                    for m in range(NCH):
                        oo_t = oo_p.tile([128, 512], F32, name="oo_t")
                        nc.scalar.mul(oo_t[:], po[m][:], ns_sb[:, m : m + 1])
                        nc.sync.dma_start(
                            out[m * 128 : (m + 1) * 128, n * 512 : (n + 1) * 512],
                            oo_t[:],
                        )
        free_gate()
        free_swm()
        free_kdm()
        free_nw()
        free_eps()
        free_ones()
        free_ones_f()
        free_ident()
    nc.finalize()
    return nc


_CACHE = {}


def _get_nc():
    if "nc" not in _CACHE:
        _CACHE["nc"] = _build_nc()
    return _CACHE["nc"]


def _host_prep(hidden_states, qkv_w, out_w, gate_w, norm_w):
    slope, qd, kd, dd, bd = _decay()
    w3 = qkv_w.reshape(H, 3 * D, HID)
    wq = w3[:, 0:D, :]
    wk = w3[:, D : 2 * D, :]
    wv = w3[:, 2 * D : 3 * D, :]
    wqk = np.concatenate([wq, wk], axis=1).reshape(2 * H * D, HID)
    wqkT = np.ascontiguousarray(wqk.T, dtype=np.float32)
    wvT = np.ascontiguousarray(wv.reshape(H * D, HID).T, dtype=np.float32)
    gwT = np.ascontiguousarray(gate_w.T, dtype=np.float32)
    owT = np.ascontiguousarray(out_w.T, dtype=np.float32)
    ddm = np.ascontiguousarray(
        dd.transpose(0, 2, 1).reshape(H, 2, 128, BS), dtype=np.float32
    )
    qdm = np.ascontiguousarray(
        np.broadcast_to(qd[:, None, :], (H, D, BS)), dtype=np.float32
    )
    kdm = np.ascontiguousarray(
        kd.reshape(H, 2, 128).transpose(2, 0, 1).reshape(128, 2 * H), dtype=np.float32
    )
    nw = np.ascontiguousarray(norm_w.reshape(16, 128).T, dtype=np.float32)

    shared = dict(wqkT=wqkT, wvT=wvT, gwT=gwT, owT=owT, ddm=ddm, qdm=qdm,
                  kdm=kdm, nw=nw)
    in_maps = []
    for c in range(NC):
        bb, p = c // 4, c % 4
        hsT = np.ascontiguousarray(
            hidden_states[bb, p * T : (p + 1) * T, :].T, dtype=np.float32
        )
        sw = np.zeros((H, NC), dtype=np.float64)
        for cc in range(NC):
            if cc // 4 == bb and cc % 4 < p:
                sw[:, cc] = bd ** (4.0 * (p - 1 - (cc % 4)))
        swm = np.ascontiguousarray(
            np.broadcast_to(sw.reshape(1, H * NC), (D, H * NC)), dtype=np.float32
        )
        in_maps.append(dict(hsT=hsT, swm=swm, **shared))
    return in_maps


def _run(inputs, trace=False):
    nc = _get_nc()
    in_maps = _host_prep(
        np.asarray(inputs["hidden_states"], dtype=np.float32),
        np.asarray(inputs["qkv_w"], dtype=np.float32),
        np.asarray(inputs["out_w"], dtype=np.float32),
        np.asarray(inputs["gate_w"], dtype=np.float32),
        np.asarray(inputs["norm_w"], dtype=np.float32),
    )
    res = run_bass_kernel_spmd(nc, in_maps, core_ids=list(range(NC)), trace=trace)
    full = np.empty((B, S, HID), dtype=np.float32)
    for c in range(NC):
        bb, p = c // 4, c % 4
        full[bb, p * T : (p + 1) * T, :] = res.results[c]["out"]
    return full, res


def kernel(**inputs):
    return _run(inputs, trace=False)[0]


def kernel_traced(**inputs):
    full, res = _run(inputs, trace=True)
    return full, res.exec_time_ns
